# revision 7
# baseline (speedup 1.0000x reference)
"""Multi-head attention (B=2, H=16, F=T=2048, N=64, HID=1024) on 8 trn2 cores.

Sharding: core c = (b, g) with b = c//4 (batch), g = c%4 (head group of 4
heads).  Each core computes, for its batch b and head-group g:
  - qhT_g = (q[b] @ Wq[:, gcols] + bq[gcols])^T          [256, F]
  - khT_g = 0.125 * (k[b] @ Wk[:, gcols] + bk[gcols])^T  [256, T]
  - vh_g  = v[b] @ Wv[:, gcols] + bv[gcols]              [T, 256] (+ ones col per head)
  - per head: S^T = khT_h^T-contract: S^T[t, f], exp with per-partition
    (= per-key) mask bias fused into the Exp activation, PV accumulated
    with an appended ones column so row 64 of the PSUM tile is the
    softmax denominator, normalization by DMA-broadcast reciprocal
  - partial output = o_g @ Wo[gcols, :]                  [F, HID]
Host sums the 4 partials per batch and adds bo.

Activations/weights feeding the PE are written as float32r (TF32-like,
full-rate); accumulation is fp32 in PSUM.  q/k/v are host-transposed so
every DMA is contiguous-per-partition.

The masks input is used to compact the key dimension: only keys with
mask==1 are shipped/computed (padded to a multiple of 128 with -1e9
exp-bias so padded keys contribute exactly 0), which is numerically
identical to the dense masked softmax.  Falls back to dense when masks
are mostly ones.
"""

import numpy as np
from contextlib import ExitStack

import concourse.tile as tile
from concourse import bacc, mybir

F32 = mybir.dt.float32
F32R = mybir.dt.float32r

H = 16
N = 64
HID = 1024
B = 2
LF = 2048
LT = 2048
NCORES = 8
GROUPS = 4          # head groups (one per core within a batch)
HPG = H // GROUPS   # heads per group = 4
GCOLS = HPG * N     # 256 hidden cols per group
P = 128

_PROGRAM_CACHE = {}


def _emit(tc, aps, lt_pad):
    """Emit the per-core tile program. lt_pad = padded key length (mult of 128)."""
    nc = tc.nc
    Exp = mybir.ActivationFunctionType.Exp
    mult = mybir.AluOpType.mult
    add = mybir.AluOpType.add
    nt = lt_pad // P          # number of key tiles
    nf = LF // 512            # f tiles of 512
    nk = HID // P             # contraction chunks

    with ExitStack() as ctx:
        # ---- persistent pools ----
        qk_pool = ctx.enter_context(tc.tile_pool(name="qk", bufs=4))
        vh_pool = ctx.enter_context(tc.tile_pool(name="vh", bufs=nt))
        oT_pool = ctx.enter_context(tc.tile_pool(name="oT", bufs=2))
        const_pool = ctx.enter_context(tc.tile_pool(name="const", bufs=1))

        maskT = const_pool.tile([P, nt], F32, tag="maskT")
        nc.sync.dma_start(maskT[:], aps["maskT"][:])
        ones128 = const_pool.tile([1, P], F32R, tag="ones")
        nc.sync.dma_start(ones128[:], aps["ones1"][:])
        bvr = const_pool.tile([1, GCOLS], F32R, tag="bvr")
        nc.sync.dma_start(bvr[:], aps["bvr"][:])
        bias_t = {}
        for nm in ("bq", "bk"):
            for m in range(2):
                t = const_pool.tile([P, 1], F32, tag=f"{nm}{m}")
                nc.sync.dma_start(
                    t[:], aps[nm][m : m + 1, :].rearrange("a b -> b a")
                )
                bias_t[(nm, m)] = t

        qh = [qk_pool.tile([P, LF], F32R, tag="qk", name=f"qh{m}") for m in range(2)]
        kh = [qk_pool.tile([P, lt_pad], F32R, tag="qk", name=f"kh{m}") for m in range(2)]
        vh = [vh_pool.tile([P, HPG * 65], F32R, tag="vh", name=f"vh{t}") for t in range(nt)]
        oT = [oT_pool.tile([P, LF], F32R, tag="oT", name=f"oT{m}") for m in range(2)]

        # ---- phase 1: projections ----
        with ExitStack() as p1:
            wpool = p1.enter_context(tc.tile_pool(name="w", bufs=16))
            xpool = p1.enter_context(tc.tile_pool(name="x", bufs=8))
            pj = p1.enter_context(tc.tile_pool(name="pj", bufs=4, space="PSUM"))

            # q and k projections -> transposed layout [256, len]
            for name, xap, wap, dest, scale, bias_nm in (
                ("q", aps["xq"], aps["wq"], qh, 1.0, "bq"),
                ("k", aps["xk"], aps["wk"], kh, 0.125, "bk"),
            ):
                ln = LF if name == "q" else lt_pad
                wt = [wpool.tile([P, GCOLS], F32R, tag="w", name=f"w{name}{kk}") for kk in range(nk)]
                for kk in range(nk):
                    nc.sync.dma_start(wt[kk][:], wap[kk * P : (kk + 1) * P, :])
                for fs in range(0, ln, 512):
                    w512 = min(512, ln - fs)
                    ps = [pj.tile([P, 512], F32, tag="pj", name=f"pj{m}") for m in range(2)]
                    for kk in range(nk):
                        xt = xpool.tile([P, 512], F32R, tag="x")
                        nc.sync.dma_start(
                            xt[:, :w512],
                            xap[kk * P : (kk + 1) * P, fs : fs + w512],
                        )
                        for m in range(2):
                            nc.tensor.matmul(
                                ps[m][:, :w512],
                                wt[kk][:, m * P : (m + 1) * P],
                                xt[:, :w512],
                                start=(kk == 0),
                                stop=(kk == nk - 1),
                            )
                    for m in range(2):
                        nc.vector.tensor_scalar(
                            dest[m][:, fs : fs + w512],
                            ps[m][:, :w512],
                            scale,
                            bias_t[(bias_nm, m)][:],
                            mult,
                            add,
                        )

            # v projection -> natural layout [lt_pad, 256] + ones col per head
            wtv = [wpool.tile([P, GCOLS], F32R, tag="w", name=f"wv{kk}") for kk in range(nk)]
            for kk in range(nk):
                nc.sync.dma_start(wtv[kk][:], aps["wv"][kk * P : (kk + 1) * P, :])
            for tg in range((nt + 3) // 4):
                ts = range(tg * 4, min(nt, tg * 4 + 4))
                width = len(ts) * P
                xts = [xpool.tile([P, 512], F32R, tag="x", name=f"xv{kk}") for kk in range(nk)]
                for kk in range(nk):
                    nc.sync.dma_start(
                        xts[kk][:, :width],
                        aps["xv"][
                            kk * P : (kk + 1) * P, tg * 512 : tg * 512 + width
                        ],
                    )
                for t in ts:
                    ps = pj.tile([P, GCOLS], F32, tag="pj")
                    for kk in range(nk):
                        lo = (t % 4) * P
                        nc.tensor.matmul(
                            ps[:],
                            xts[kk][:, lo : lo + P],
                            wtv[kk][:],
                            start=(kk == 0),
                            stop=False,
                        )
                    nc.tensor.matmul(
                        ps[:], ones128[:], bvr[:], start=False, stop=True
                    )
                    view = vh[t][:].rearrange("p (h x) -> p h x", h=HPG)
                    nc.vector.tensor_copy(
                        view[:, :, 0:64],
                        ps[:].rearrange("p (h n) -> p h n", h=HPG),
                    )
                    nc.sync.dma_start(view[:, :, 64:65], aps["ones4"][:])

        # ---- phase 2: attention ----
        with ExitStack() as p2:
            epool = p2.enter_context(tc.tile_pool(name="e", bufs=4))
            rpool = p2.enter_context(tc.tile_pool(name="r", bufs=2))
            bpool = p2.enter_context(tc.tile_pool(name="bc", bufs=2))
            dpool = p2.enter_context(tc.tile_pool(name="rcd", bufs=4, space="DRAM"))
            psS = p2.enter_context(tc.tile_pool(name="psS", bufs=2, space="PSUM"))
            psO = p2.enter_context(tc.tile_pool(name="psO", bufs=4, space="PSUM"))

            for h in range(HPG):
                ch, off = h // 2, (h % 2) * 64
                for f2 in range(nf // 2):
                    fbase = f2 * 1024
                    acc = [psO.tile([65, 512], F32, tag="o", name=f"acc{fh}") for fh in range(2)]
                    for t in range(nt):
                        s = psS.tile([P, 1024], F32, tag="s")
                        for fh in range(2):
                            nc.tensor.matmul(
                                s[:, fh * 512 : (fh + 1) * 512],
                                kh[ch][off : off + 64, t * P : (t + 1) * P],
                                qh[ch][
                                    off : off + 64,
                                    fbase + fh * 512 : fbase + (fh + 1) * 512,
                                ],
                                start=True,
                                stop=True,
                            )
                        e = epool.tile([P, 1024], F32R, tag="e")
                        nc.scalar.activation(
                            e[:], s[:], Exp, bias=maskT[:, t : t + 1], scale=1.0
                        )
                        vview = vh[t][:].rearrange("p (h x) -> p h x", h=HPG)
                        for fh in range(2):
                            nc.tensor.matmul(
                                acc[fh][:],
                                vview[:, h, :],
                                e[:, fh * 512 : (fh + 1) * 512],
                                start=(t == 0),
                                stop=(t == nt - 1),
                            )
                    rc = rpool.tile([1, 1024], F32, tag="rc")
                    for fh in range(2):
                        nc.vector.reciprocal(
                            rc[0:1, fh * 512 : (fh + 1) * 512], acc[fh][64:65, :]
                        )
                    rd = dpool.tile([1, 1024], F32, tag="rcd")
                    nc.sync.dma_start(rd[:], rc[0:1, :])
                    bc = bpool.tile([64, 1024], F32, tag="bc")
                    nc.sync.dma_start(bc[:], rd[:].to_broadcast((64, 1024)))
                    for fh in range(2):
                        nc.vector.tensor_mul(
                            oT[ch][
                                off : off + 64,
                                fbase + fh * 512 : fbase + (fh + 1) * 512,
                            ],
                            acc[fh][0:64, :],
                            bc[:, fh * 512 : (fh + 1) * 512],
                        )

        # ---- phase 3: output projection (partial) ----
        with ExitStack() as p3:
            wopool = p3.enter_context(tc.tile_pool(name="wo", bufs=2))
            opool = p3.enter_context(tc.tile_pool(name="os", bufs=4))
            psW = p3.enter_context(tc.tile_pool(name="psW", bufs=4, space="PSUM"))

            wot = [wopool.tile([P, HID], F32R, tag="wo", name=f"wot{kk}") for kk in range(2)]
            for kk in range(2):
                nc.sync.dma_start(wot[kk][:], aps["wo"][kk * P : (kk + 1) * P, :])
            for ft in range(LF // P):
                ot = opool.tile([P, HID], F32, tag="os")
                for half in range(2):
                    ps = psW.tile([P, 512], F32, tag="w")
                    for kk in range(2):
                        nc.tensor.matmul(
                            ps[:],
                            oT[kk][:, ft * P : (ft + 1) * P],
                            wot[kk][:, half * 512 : (half + 1) * 512],
                            start=(kk == 0),
                            stop=(kk == 1),
                        )
                    nc.vector.tensor_copy(ot[:, half * 512 : (half + 1) * 512], ps[:])
                nc.sync.dma_start(aps["out"][ft * P : (ft + 1) * P, :], ot[:])


def _build_program(lt_pad):
    if lt_pad in _PROGRAM_CACHE:
        return _PROGRAM_CACHE[lt_pad]
    nc = bacc.Bacc("TRN2", target_bir_lowering=False, debug=False)
    aps = {
        "xq": nc.dram_tensor("xq", [HID, LF], F32R, kind="ExternalInput").ap(),
        "xk": nc.dram_tensor("xk", [HID, lt_pad], F32R, kind="ExternalInput").ap(),
        "xv": nc.dram_tensor("xv", [HID, lt_pad], F32R, kind="ExternalInput").ap(),
        "wq": nc.dram_tensor("wq", [HID, GCOLS], F32R, kind="ExternalInput").ap(),
        "wk": nc.dram_tensor("wk", [HID, GCOLS], F32R, kind="ExternalInput").ap(),
        "wv": nc.dram_tensor("wv", [HID, GCOLS], F32R, kind="ExternalInput").ap(),
        "wo": nc.dram_tensor("wo", [GCOLS, HID], F32R, kind="ExternalInput").ap(),
        "bq": nc.dram_tensor("bq", [2, P], F32, kind="ExternalInput").ap(),
        "bk": nc.dram_tensor("bk", [2, P], F32, kind="ExternalInput").ap(),
        "bvr": nc.dram_tensor("bvr", [1, GCOLS], F32R, kind="ExternalInput").ap(),
        "maskT": nc.dram_tensor(
            "maskT", [P, lt_pad // P], F32, kind="ExternalInput"
        ).ap(),
        "ones1": nc.dram_tensor("ones1", [1, P], F32R, kind="ExternalInput").ap(),
        "ones4": nc.dram_tensor(
            "ones4", [P, HPG, 1], F32R, kind="ExternalInput"
        ).ap(),
        "out": nc.dram_tensor("out", [LF, HID], F32, kind="ExternalOutput").ap(),
    }
    with tile.TileContext(nc) as tc:
        _emit(tc, aps, lt_pad)
    nc.compile()
    _PROGRAM_CACHE[lt_pad] = nc
    return nc


def _choose_lt_pad(masks):
    keep = [np.nonzero(np.asarray(masks)[b] != 0)[0] for b in range(B)]
    max_keep = max(len(ix) for ix in keep)
    lt_pad = max(P, ((max_keep + P - 1) // P) * P)
    if lt_pad >= LT:
        lt_pad = LT
    return lt_pad, keep


def _make_in_maps(inputs, lt_pad):
    q = np.asarray(inputs["q"], np.float32)
    k = np.asarray(inputs["k"], np.float32)
    v = np.asarray(inputs["v"], np.float32)
    masks = np.asarray(inputs["masks"])
    Wq, bq = np.asarray(inputs["Wq"], np.float32), np.asarray(inputs["bq"], np.float32)
    Wk, bk = np.asarray(inputs["Wk"], np.float32), np.asarray(inputs["bk"], np.float32)
    Wv, bv = np.asarray(inputs["Wv"], np.float32), np.asarray(inputs["bv"], np.float32)
    Wo = np.asarray(inputs["Wo"], np.float32)

    _, keep = _choose_lt_pad(masks)
    compact = lt_pad < LT
    in_maps = []
    for c in range(NCORES):
        b, g = c // GROUPS, c % GROUPS
        gs = slice(g * GCOLS, (g + 1) * GCOLS)
        if compact:
            ix = keep[b]
            kb = np.zeros((lt_pad, HID), np.float32)
            vb = np.zeros((lt_pad, HID), np.float32)
            kb[: len(ix)] = k[b][ix]
            vb[: len(ix)] = v[b][ix]
            mbias = np.full(lt_pad, -1e9, np.float32)
            mbias[: len(ix)] = 0.0
        else:
            kb, vb = k[b], v[b]
            mbias = (masks[b].astype(np.float32) - 1.0) * 1e9
        in_maps.append(
            {
                "xq": np.ascontiguousarray(q[b].T),
                "xk": np.ascontiguousarray(kb.T),
                "xv": np.ascontiguousarray(vb.T),
                "wq": np.ascontiguousarray(Wq[:, gs]),
                "wk": np.ascontiguousarray(Wk[:, gs]),
                "wv": np.ascontiguousarray(Wv[:, gs]),
                "wo": np.ascontiguousarray(Wo[gs, :]),
                "bq": np.asarray(bq[gs], np.float32).reshape(2, P),
                "bk": (np.asarray(bk[gs], np.float32) * 0.125).reshape(2, P),
                "bvr": np.asarray(bv[gs], np.float32).reshape(1, GCOLS),
                "maskT": np.ascontiguousarray(
                    mbias.reshape(lt_pad // P, P).T
                ),
                "ones1": np.ones((1, P), np.float32),
                "ones4": np.ones((P, HPG, 1), np.float32),
            }
        )
    return in_maps


def kernel(q, k, v, masks, Wq, bq, Wk, bk, Wv, bv, Wo, bo):
    from concourse.bass_utils import run_bass_kernel_spmd

    assert np.asarray(q).shape == (B, LF, HID)
    lt_pad, _ = _choose_lt_pad(masks)
    nc = _build_program(lt_pad)
    in_maps = _make_in_maps(
        dict(q=q, k=k, v=v, masks=masks, Wq=Wq, bq=bq, Wk=Wk, bk=bk,
             Wv=Wv, bv=bv, Wo=Wo, bo=bo),
        lt_pad,
    )

    res = run_bass_kernel_spmd(nc, in_maps, core_ids=list(range(NCORES)))
    out = np.zeros((B, LF, HID), np.float32)
    for c in range(NCORES):
        out[c // GROUPS] += res.results[c]["out"]
    out += np.asarray(bo, np.float32)
    return out
